# revision 28
# baseline (speedup 1.0000x reference)
"""Kernel library for MultiHeadDoubleAttention on TRN2 (v3, all-bf16).

Data-parallel over batch: 8 cores x 16 batch each.

Conv formulation: 15x15 conv with pad 7 on an 8x8 grid == sum over 65
non-masked taps (shifts s=(sr,sc)) of channel-matmuls applied to
shifted pixel rectangles:
    y[o, b, i] += sum_c x[c, b, i+s] * w_tap[c, o]
Matmul: lhsT = w_tap [c=128 (K), o=128 (M)] slice, rhs = x [c, (rect, b)]
(N = 16*rect), accumulating into psum [128, 4, 8, 16] (one bank per
(o_half, row_half)).

All bf16 (weights + activations): PE fast weight load stays hidden under
the matmul stream; fp32 PSUM accumulate; rel err ~1.5e-3 vs 2e-2 gate.

v3 scheduling:
 - inputs DMA'd on the gpsimd queue in parallel with weight chunks (sync).
 - P3 (k2+q2, shared resident wk) split into oh=0 / oh=1 half-passes;
   each half's copy-outs run split across scalar (kh) + vector (qh).
 - V transposes for attention are DMA XBAR transposes (bf16) issued
   during P3 -- zero tensor-engine cost.
 - Attention psum tiles reuse the conv psum pool tags, so there is no
   pool-boundary barrier; scores matmuls are emitted batch-major for
   8-way (4 row x 2 col) PE tile concurrency.
 - Softmax normalization is 2 DVE ops per batch-pair (reciprocal +
   broadcast multiply); output projection pipelines per batch-pair.
"""
import sys
sys.path.insert(0, '/opt/trn_rl_repo')
import numpy as np
import ml_dtypes

import concourse.bass as bass
import concourse.bacc as bacc
import concourse.mybir as mybir
import concourse.bass_utils as bass_utils
from concourse.tile import TileContext

F32 = mybir.dt.float32
BF16 = mybir.dt.bfloat16

B, D, H, DK = 128, 256, 8, 32
NCORES = 8
BL = B // NCORES          # batch per core
NPIX = 64                 # 8x8
NB2 = BL // 2             # 8 batch pairs
NTAPS = 65
RS = 1.0 / np.sqrt(DK)    # score scale


def hollow_mask():
    m = np.ones((15, 15), np.float32)
    for c in range(5):
        m[1 + c:7, c] = 0; m[8:14 - c, c] = 0
        m[c, 1 + c:7] = 0; m[c, 8:14 - c] = 0
        m[1 + c:7, 14 - c] = 0; m[8:14 - c, 14 - c] = 0
        m[14 - c, 1 + c:7] = 0; m[14 - c, 8:14 - c] = 0
    return m


def tap_schedule():
    """All 65 unmasked taps as (sr, sc, ar0, hr, ac0, wc), biggest first."""
    m = hollow_mask()
    taps = []
    for di in range(15):
        for dj in range(15):
            if not m[di, dj]:
                continue
            sr, sc = di - 7, dj - 7
            ar0, ar1 = max(0, sr), min(7, 7 + sr)
            ac0, ac1 = max(0, sc), min(7, 7 + sc)
            taps.append((sr, sc, ar0, ar1 - ar0 + 1, ac0, ac1 - ac0 + 1))
    taps.sort(key=lambda e: -(e[3] * e[5]))
    return taps


def _deal_taps(taps):
    """Reorder taps so streamed chunks carry balanced compute: chunk 0 is
    the 2 biggest taps (fast start); the remaining 63 are dealt round-robin
    (big..small) into 7 chunks of 9 so chunk DMA time never exceeds chunk
    compute time at the tail of a pass."""
    head, rest = taps[:2], taps[2:]
    piles = [[] for _ in range(7)]
    for i, t in enumerate(rest):
        piles[i % 7].append(t)
    out = list(head)
    for p in piles:
        out.extend(p)
    return out


TAPS = _deal_taps(tap_schedule())
CHUNKS = [2] + [9] * 7


def tap_pieces(sr, sc, ar0, hr, ac0, wc):
    """Split a tap's output rect at the ir=4 psum-bank boundary.
    Returns list of (bank, ir0_in_bank, ar0, ph, ic0, ac0, wc)."""
    ir0 = ar0 - sr
    ic0 = ac0 - sc
    pieces = []
    lo, hi = ir0, ir0 + hr
    if lo < 4:
        ph = min(hi, 4) - lo
        pieces.append((0, lo, lo + sr, ph, ic0, ac0, wc))
    if hi > 4:
        p0 = max(lo, 4)
        ph = hi - p0
        pieces.append((1, p0 - 4, p0 + sr, ph, ic0, ac0, wc))
    return pieces


PIECES = [tap_pieces(*t) for t in TAPS]


def prep_weights(w):
    """w: [D, D, 15, 15] OIHW -> bf16 [NTAPS, 2, 128, 256] laid out [c, o]."""
    wt = np.empty((NTAPS, 2, 128, 256), ml_dtypes.bfloat16)
    for i, (sr, sc, *_r) in enumerate(TAPS):
        wt[i] = w[:, :, sr + 7, sc + 7].T.reshape(2, 128, 256).astype(
            ml_dtypes.bfloat16)
    return wt


def stream_weights(nc, pool, w_dram, tag):
    """Rotating-chunk DMAs of one conv weight tensor, alternating between
    the sync and gpsimd DMA queues so chunk transfers overlap."""
    chunks = []
    c0 = 0
    for i, n in enumerate(CHUNKS):
        wt = pool.tile([128, max(CHUNKS), 2, 256], BF16, tag=f"{tag}wS",
                       name=f"{tag}wS{c0}")
        eng = nc.sync if i % 2 == 0 else nc.gpsimd
        eng.dma_start(wt[:, :n],
                      w_dram[c0:c0 + n].rearrange("t h c o -> c t h o"))
        chunks.append((c0, n, wt[:, :n]))
        c0 += n
    return chunks


def resident_weights(nc, pool, w_dram, tag, chunk=8):
    """One SBUF-resident tile holding all taps (chunked DMAs)."""
    wt = pool.tile([128, NTAPS, 2, 256], BF16, tag=f"{tag}wR", name=f"{tag}wR")
    chunks = []
    for c0 in range(0, NTAPS, chunk):
        n = min(chunk, NTAPS - c0)
        nc.sync.dma_start(wt[:, c0:c0 + n],
                          w_dram[c0:c0 + n].rearrange("t h c o -> c t h o"))
        chunks.append((c0, n, wt[:, c0:c0 + n]))
    return chunks


def conv_pass(nc, psum_pool, chunks, inputs, tag, ptag, ohs=(0, 1),
              filler=None):
    """inputs: list of [xh0, xh1] bf16 tiles [128, 8, 8, BL]."""
    ptags = [ptag] * len(inputs) if isinstance(ptag, str) else ptag
    ps = [{oh: [psum_pool.tile([128, 4, 8, BL], F32,
                               tag=f"{ptags[ii]}ps{oh}{bk}",
                               name=f"{tag}ps{ii}{oh}{bk}")
                for bk in range(2)] for oh in ohs}
          for ii in range(len(inputs))]
    total = {0: 0, 1: 0}
    for pcs in PIECES:
        for p in pcs:
            total[p[0]] += 2            # x2 c_halves
    done = {}
    for c0, n, wt in chunks:
        for tl in range(n):
            pieces = PIECES[c0 + tl]
            for oh in ohs:
                for ch in range(2):
                    lhsT = wt[:, tl, ch, oh * 128:(oh + 1) * 128]
                    for ii, xh in enumerate(inputs):
                        for (bk, irb, ar0, ph, ic0, ac0, wc) in pieces:
                            key = (ii, oh, bk)
                            cnt = done.get(key, 0)
                            done[key] = cnt + 1
                            rhs = xh[ch][:, ar0:ar0 + ph, ac0:ac0 + wc, :]
                            out = ps[ii][oh][bk][:, irb:irb + ph,
                                                 ic0:ic0 + wc, :]
                            nc.tensor.matmul(out, lhsT, rhs,
                                             start=(cnt == 0),
                                             stop=(cnt == total[bk] - 1))
        if filler is not None:
            filler(c0)
    return ps


def build_kernel():
    """Build the full per-core kernel (same NEFF on all 8 cores)."""
    nc = bacc.Bacc("TRN2", target_bir_lowering=False, debug=False,
                   num_devices=NCORES)
    dt = {}
    for nmm in ("q", "k", "v"):
        dt[f"x{nmm}"] = nc.dram_tensor(f"x{nmm}", [2, 128, 8, 8, BL], BF16,
                                       kind="ExternalInput")
        dt[f"w{nmm}"] = nc.dram_tensor(f"w{nmm}", [NTAPS, 2, 128, 256], BF16,
                                       kind="ExternalInput")
        dt[f"bias{nmm}"] = nc.dram_tensor(f"bias{nmm}", [2, 128], F32,
                                          kind="ExternalInput")
    dt["wo_t"] = nc.dram_tensor("wo_t", [2, 128, 256], BF16,
                                kind="ExternalInput")
    dt["bo"] = nc.dram_tensor("bo", [1, 256], BF16, kind="ExternalInput")
    dt["ones"] = nc.dram_tensor("ones", [1, 128], BF16, kind="ExternalInput")
    dt["ident"] = nc.dram_tensor("ident", [128, 128], BF16,
                                 kind="ExternalInput")
    dt["out"] = nc.dram_tensor("out", [8, 128, 256], F32,
                               kind="ExternalOutput")

    with TileContext(nc) as tc:
      with tc.tile_pool(name="persist", bufs=1) as pp:
        bias_t = {}
        for nmm in ("q", "k", "v"):
            bias_t[nmm] = pp.tile([128, 2], F32, name=f"bias{nmm}_t")
        ones_t = pp.tile([1, 128], BF16, name="ones_t")
        bo_t = pp.tile([1, 256], BF16, name="bo_t")
        ident_t = pp.tile([128, 128], BF16, name="ident_t")
        wo_tt = [pp.tile([128, 256], BF16, name=f"wo_tt{h}") for h in range(2)]

        def emit_persist_dmas():
            # on the gpsimd queue: off the weight-stream critical path
            for nmm in ("q", "k", "v"):
                nc.gpsimd.dma_start(bias_t[nmm][:],
                                    dt[f"bias{nmm}"].ap().rearrange(
                                        "h c -> c h"))
            nc.gpsimd.dma_start(ones_t[:], dt["ones"][:])
            nc.gpsimd.dma_start(bo_t[:], dt["bo"][:])
            nc.gpsimd.dma_start(ident_t[:], dt["ident"][:])
            for h in range(2):
                nc.gpsimd.dma_start(wo_tt[h][:], dt["wo_t"][h])

        # conv2 outputs [o, b, pix] -- live into attention
        hh = {}
        for nmm in ("q", "k", "v"):
            hh[nmm] = [pp.tile([128, BL, NPIX], BF16, name=f"h{nmm}{h}")
                       for h in range(2)]
        # inputs and conv1 outputs, all bf16
        xx, x1 = {}, {}
        for nmm in ("q", "k", "v"):
            xx[nmm] = [pp.tile([128, 8, 8, BL], BF16, name=f"x{nmm}{h}")
                       for h in range(2)]
            x1[nmm] = [pp.tile([128, 8, 8, BL], BF16, name=f"x1{nmm}{h}")
                       for h in range(2)]
        # attention SBUF tiles
        E_t = pp.tile([128, NB2, H, NPIX], BF16, name="E_t")
        VT = pp.tile([128, NB2, H, 33], BF16, name="VT")
        OA2 = pp.tile([128, NB2, 2, 128], BF16, name="OA2")
        concat = [pp.tile([128, BL, NPIX], BF16, name=f"concat{h}")
                  for h in range(2)]
        rcp = pp.tile([128, NB2, H], F32, name="rcp")
        out_sb = pp.tile([128, 8, 256], F32, name="out_sb")

        def load_x(nmm):
            for h in range(2):
                nc.gpsimd.dma_start(xx[nmm][h][:], dt[f"x{nmm}"][h])

        def copy_out_relu(ps, x1t, bias):
            # split across scalar and vector so the next pass's first
            # matmuls (which the scheduler may hoist) wait ~half as long
            for oh in range(2):
                for bk in range(2):
                    dst = x1t[oh][:, bk * 4:(bk + 1) * 4, :, :]
                    if bk == 0:
                        nc.scalar.activation(
                            dst, ps[oh][bk][:],
                            mybir.ActivationFunctionType.Relu,
                            bias=bias[:, oh:oh + 1])
                    else:
                        nc.vector.tensor_scalar(
                            dst, ps[oh][bk][:], bias[:, oh:oh + 1], 0.0,
                            mybir.AluOpType.add, mybir.AluOpType.max)

        def copy_out_final(ps_oh, out_t_oh, bias, oh, engine="scalar"):
            for bk in range(2):
                src = ps_oh[bk][:].rearrange("c pr pc b -> c b (pr pc)")
                dst = out_t_oh[:, :, bk * 32:(bk + 1) * 32]
                if engine == "scalar":
                    nc.scalar.activation(
                        dst, src, mybir.ActivationFunctionType.Identity,
                        bias=bias[:, oh:oh + 1])
                else:
                    nc.vector.tensor_scalar_add(dst, src, bias[:, oh:oh + 1])

        with tc.tile_pool(name="cvps", bufs=1, space="PSUM") as cvpp:
            # ---- P1: q -> q1 (stream wq) ----
            with tc.tile_pool(name="wsq", bufs=4) as wsp:
                load_x('q')
                chq = stream_weights(nc, wsp, dt["wq"].ap(), "q")
                emit_persist_dmas()
                ps = conv_pass(nc, cvpp, chq, [xx['q']], tag="p1", ptag="cvA")
                copy_out_relu(ps[0], x1['q'], bias_t['q'])

            with tc.tile_pool(name="wkres", bufs=1) as wkp:
                # ---- P4: v -> v1 (stream wv); prefetch resident wk ----
                with tc.tile_pool(name="wsv1", bufs=4) as wsv:
                    load_x('v')
                    chv = stream_weights(nc, wsv, dt["wv"].ap(), "v1")
                    chk = resident_weights(nc, wkp, dt["wk"].ap(), "k")
                    load_x('k')
                    ps = conv_pass(nc, cvpp, chv, [xx['v']], tag="p4",
                                   ptag="cvB")
                    copy_out_relu(ps[0], x1['v'], bias_t['v'])

                # ---- P2: k -> k1 (wk resident) ----
                ps = conv_pass(nc, cvpp, chk, [xx['k']], tag="p2", ptag="cvA")
                copy_out_relu(ps[0], x1['k'], bias_t['k'])

                # ---- P5: v1 -> vh (re-stream wv) ----
                with tc.tile_pool(name="wsv2", bufs=4) as wsv2:
                    chv2 = stream_weights(nc, wsv2, dt["wv"].ap(), "v2")
                    ps = conv_pass(nc, cvpp, chv2, [x1['v']], tag="p5",
                                   ptag="cvB")
                    for oh in range(2):
                        copy_out_final(ps[0][oh], hh['v'][oh], bias_t['v'], oh)

                # V transposes for attention: DMA XBAR (runs during P3).
                # in vh[oh][:, 2b2:2b2+2, :] = [128=(4h,32dk), (2b,64pix)];
                # out partitions = (par, kpix), free = (h, dk).  The XBAR
                # needs a contiguous 2D dst, so stage then strided-copy.
                nc.vector.memset(VT[:, :, :, 32:33], 1.0)
                vstage = pp.tile([128, NB2, 2, 128], BF16, name="vstage")
                for b2 in range(NB2):
                    for oh in range(2):
                        nc.sync.dma_start_transpose(
                            vstage[:, b2, oh, :],
                            hh['v'][oh][:, 2 * b2:2 * b2 + 2, :].rearrange(
                                "c b p -> c (b p)"))
                        nc.vector.tensor_copy(
                            VT[:, b2, oh * 4:(oh + 1) * 4, 0:32],
                            vstage[:, b2, oh, :].rearrange(
                                "p (h d) -> p h d", d=32))

                # ---- P3: {k1, q1} -> {kh, qh}; oh halves split so the
                # first half's copy-outs + scores overlap the second half.
                kh, qh = hh['k'], hh['q']
                pst = {}
                PTAG = ["cvAps00", "cvAps01", "cvBps00", "cvBps01",
                        "cvAps10", "cvAps11", "cvBps10", "cvBps11"]

                def scores_half(oh, tags):
                    for hp in range(4):
                        h = oh * 4 + hp
                        pst[h] = cvpp.tile([128, NB2, 64], F32, tag=tags[hp],
                                           name=f"pst{h}")
                    for half in range(2):
                        for b2 in range(half * 4, half * 4 + 4):
                            for par in range(2):
                                b = 2 * b2 + par
                                for hp in range(4):
                                    h = oh * 4 + hp
                                    nc.tensor.matmul(
                                        pst[h][64 * par:64 * par + 64, b2, :],
                                        kh[oh][hp * 32:(hp + 1) * 32, b, :],
                                        qh[oh][hp * 32:(hp + 1) * 32, b, :],
                                        start=True, stop=True,
                                        tile_position=(32 * hp, 64 * par))
                        for hp in range(4):
                            h = oh * 4 + hp
                            nc.scalar.activation(
                                E_t[:, half * 4:half * 4 + 4, h, :],
                                pst[h][:, half * 4:half * 4 + 4, :],
                                mybir.ActivationFunctionType.Exp, scale=RS)

                ps = conv_pass(nc, cvpp, chk, [x1['k'], x1['q']], tag="p3a",
                               ptag=["cvA", "cvB"], ohs=(0,))
                copy_out_final(ps[0][0], kh[0], bias_t['k'], 0, "scalar")
                copy_out_final(ps[1][0], qh[0], bias_t['k'], 0, "vector")
                scores_half(0, PTAG[0:4])

                ps = conv_pass(nc, cvpp, chk, [x1['k'], x1['q']], tag="p3b",
                               ptag=["cvA", "cvB"], ohs=(1,))
                copy_out_final(ps[0][1], kh[1], bias_t['k'], 1, "scalar")
                copy_out_final(ps[1][1], qh[1], bias_t['k'], 1, "vector")
                scores_half(1, PTAG[4:8])

                # ---- attention AV + normalize + transpose + projection,
                # pipelined per batch-pair ----
                cslice = [concat[oh].rearrange("c b p -> c (b p)")
                          for oh in range(2)]

                def av(b2):
                    pso = cvpp.tile([128, H, 33], F32, tag=PTAG[b2 % 4],
                                    name=f"pso{b2}")
                    for h in range(H):
                        for par in range(2):
                            nc.tensor.matmul(
                                pso[64 * par:64 * par + 64, h, :],
                                E_t[64 * par:64 * par + 64, b2, h, :],
                                VT[64 * par:64 * par + 64, b2, h, :],
                                start=True, stop=True)
                    nc.vector.reciprocal(
                        rcp[:, b2, :],
                        pso[:, :, 32:33].rearrange("p h one -> p (h one)"))
                    nc.vector.tensor_mul(
                        OA2[:, b2, :, :].rearrange(
                            "p oh (hp d) -> p (oh hp) d", d=32),
                        pso[:, :, 0:32],
                        rcp[:, b2, :].unsqueeze(-1).broadcast_to(
                            [128, H, 32]))

                def out_pair(b2):
                    for oh in range(2):
                        pot = cvpp.tile([128, 128], BF16,
                                        tag=PTAG[4 + (2 * b2 + oh) % 2],
                                        name=f"pot{b2}{oh}")
                        nc.tensor.transpose(pot[:], OA2[:, b2, oh, :],
                                            ident_t[:])
                        nc.vector.tensor_copy(
                            concat[oh][:, 2 * b2:2 * b2 + 2, :].rearrange(
                                "c b p -> c (b p)"),
                            pot[:])
                    pspr = cvpp.tile([128, 256], F32, tag=PTAG[6 + b2 % 2],
                                     name=f"pspr{b2}")
                    for oh in range(2):
                        nc.tensor.matmul(
                            pspr[:], cslice[oh][:, b2 * 128:(b2 + 1) * 128],
                            wo_tt[oh][:], start=(oh == 0), stop=False)
                    nc.tensor.matmul(pspr[:], ones_t[:], bo_t[:],
                                     start=False, stop=True)
                    nc.vector.tensor_copy(out_sb[:, b2, :], pspr[:])
                    nc.sync.dma_start(dt["out"][b2], out_sb[:, b2, :])

                for b2 in range(NB2):
                    av(b2)
                    if b2 >= 1:
                        out_pair(b2 - 1)
                out_pair(NB2 - 1)
    nc.compile()
    return nc


def prep_static(wk, bk, wq, bq, wv, bv, wo, bo):
    """Host-side weight prep shared by all cores."""
    st = {}
    for nmm, w, b in (("q", wq, bq), ("k", wk, bk), ("v", wv, bv)):
        st[f"w{nmm}"] = prep_weights(np.asarray(w, np.float32))
        st[f"bias{nmm}"] = np.ascontiguousarray(
            np.asarray(b, np.float32).reshape(2, 128))
    st["wo_t"] = np.ascontiguousarray(
        np.asarray(wo, np.float32).T).reshape(2, 128, 256).astype(
        ml_dtypes.bfloat16)
    st["bo"] = np.asarray(bo, np.float32).reshape(1, 256).astype(
        ml_dtypes.bfloat16)
    st["ones"] = np.ones((1, 128), ml_dtypes.bfloat16)
    st["ident"] = np.eye(128, dtype=np.float32).astype(ml_dtypes.bfloat16)
    return st


def prep_core_x(x, core):
    """x: [B, 8, 8, D] -> this core's bf16 [2, 128, 8, 8, BL]."""
    xs = np.asarray(x[core * BL:(core + 1) * BL], np.float32)
    return np.ascontiguousarray(xs.transpose(3, 1, 2, 0)).reshape(
        2, 128, 8, 8, BL).astype(ml_dtypes.bfloat16)


def make_in_maps(q, k, v, st):
    in_maps = []
    for core in range(NCORES):
        m = dict(st)
        m["xq"] = prep_core_x(q, core)
        m["xk"] = prep_core_x(k, core)
        m["xv"] = prep_core_x(v, core)
        in_maps.append(m)
    return in_maps


def gather_out(results):
    """results: list of dicts with 'out' [8, 128, 256] -> [B, 8, 8, D]."""
    outs = [r["out"].reshape(BL, 8, 8, D) for r in results]
    return np.concatenate(outs, axis=0)


# ---------------------------------------------------------------------------
# Self-contained entry point: kernel(**inputs) -> full [128, 8, 8, 256] output
# ---------------------------------------------------------------------------
_NC_CACHE = None


def _get_nc():
    global _NC_CACHE
    if _NC_CACHE is None:
        _NC_CACHE = build_kernel()
    return _NC_CACHE


def kernel(q, k, v, wk, bk, wq, bq, wv, bv, wo, bo):
    nc = _get_nc()
    st = prep_static(wk, bk, wq, bq, wv, bv, wo, bo)
    in_maps = make_in_maps(np.asarray(q), np.asarray(k), np.asarray(v), st)
    res = bass_utils.run_bass_kernel_spmd(
        nc, in_maps, core_ids=list(range(NCORES)))
    return gather_out(res.results)


# revision 29
# speedup vs baseline: 1.0143x; 1.0143x over previous
"""Kernel library for MultiHeadDoubleAttention on TRN2 (v3, all-bf16).

Data-parallel over batch: 8 cores x 16 batch each.

Conv formulation: 15x15 conv with pad 7 on an 8x8 grid == sum over 65
non-masked taps (shifts s=(sr,sc)) of channel-matmuls applied to
shifted pixel rectangles:
    y[o, b, i] += sum_c x[c, b, i+s] * w_tap[c, o]
Matmul: lhsT = w_tap [c=128 (K), o=128 (M)] slice, rhs = x [c, (rect, b)]
(N = 16*rect), accumulating into psum [128, 4, 8, 16] (one bank per
(o_half, row_half)).

All bf16 (weights + activations): PE fast weight load stays hidden under
the matmul stream; fp32 PSUM accumulate; rel err ~1.5e-3 vs 2e-2 gate.

v3 scheduling:
 - inputs DMA'd on the gpsimd queue in parallel with weight chunks (sync).
 - P3 (k2+q2, shared resident wk) split into oh=0 / oh=1 half-passes;
   each half's copy-outs run split across scalar (kh) + vector (qh).
 - V transposes for attention are DMA XBAR transposes (bf16) issued
   during P3 -- zero tensor-engine cost.
 - Attention psum tiles reuse the conv psum pool tags, so there is no
   pool-boundary barrier; scores matmuls are emitted batch-major for
   8-way (4 row x 2 col) PE tile concurrency.
 - Softmax normalization is 2 DVE ops per batch-pair (reciprocal +
   broadcast multiply); output projection pipelines per batch-pair.
"""
import sys
sys.path.insert(0, '/opt/trn_rl_repo')
import numpy as np
import ml_dtypes

import concourse.bass as bass
import concourse.bacc as bacc
import concourse.mybir as mybir
import concourse.bass_utils as bass_utils
from concourse.tile import TileContext

F32 = mybir.dt.float32
BF16 = mybir.dt.bfloat16

B, D, H, DK = 128, 256, 8, 32
NCORES = 8
BL = B // NCORES          # batch per core
NPIX = 64                 # 8x8
NB2 = BL // 2             # 8 batch pairs
NTAPS = 65
RS = 1.0 / np.sqrt(DK)    # score scale


def hollow_mask():
    m = np.ones((15, 15), np.float32)
    for c in range(5):
        m[1 + c:7, c] = 0; m[8:14 - c, c] = 0
        m[c, 1 + c:7] = 0; m[c, 8:14 - c] = 0
        m[1 + c:7, 14 - c] = 0; m[8:14 - c, 14 - c] = 0
        m[14 - c, 1 + c:7] = 0; m[14 - c, 8:14 - c] = 0
    return m


def tap_schedule():
    """All 65 unmasked taps as (sr, sc, ar0, hr, ac0, wc), biggest first."""
    m = hollow_mask()
    taps = []
    for di in range(15):
        for dj in range(15):
            if not m[di, dj]:
                continue
            sr, sc = di - 7, dj - 7
            ar0, ar1 = max(0, sr), min(7, 7 + sr)
            ac0, ac1 = max(0, sc), min(7, 7 + sc)
            taps.append((sr, sc, ar0, ar1 - ar0 + 1, ac0, ac1 - ac0 + 1))
    taps.sort(key=lambda e: -(e[3] * e[5]))
    return taps


def _deal_taps(taps):
    """Reorder taps so streamed chunks carry balanced compute: chunk 0 is
    the 2 biggest taps (fast start); the remaining 63 are dealt round-robin
    (big..small) into 7 chunks of 9 so chunk DMA time never exceeds chunk
    compute time at the tail of a pass."""
    head, rest = taps[:2], taps[2:]
    piles = [[] for _ in range(7)]
    for i, t in enumerate(rest):
        piles[i % 7].append(t)
    out = list(head)
    for p in piles:
        out.extend(p)
    return out


TAPS = _deal_taps(tap_schedule())
CHUNKS = [2] + [9] * 7


def tap_pieces(sr, sc, ar0, hr, ac0, wc):
    """Split a tap's output rect at the ir=4 psum-bank boundary.
    Returns list of (bank, ir0_in_bank, ar0, ph, ic0, ac0, wc)."""
    ir0 = ar0 - sr
    ic0 = ac0 - sc
    pieces = []
    lo, hi = ir0, ir0 + hr
    if lo < 4:
        ph = min(hi, 4) - lo
        pieces.append((0, lo, lo + sr, ph, ic0, ac0, wc))
    if hi > 4:
        p0 = max(lo, 4)
        ph = hi - p0
        pieces.append((1, p0 - 4, p0 + sr, ph, ic0, ac0, wc))
    return pieces


PIECES = [tap_pieces(*t) for t in TAPS]


def prep_weights(w):
    """w: [D, D, 15, 15] OIHW -> bf16 [NTAPS, 2, 128, 256] laid out [c, o]."""
    wt = np.empty((NTAPS, 2, 128, 256), ml_dtypes.bfloat16)
    for i, (sr, sc, *_r) in enumerate(TAPS):
        wt[i] = w[:, :, sr + 7, sc + 7].T.reshape(2, 128, 256).astype(
            ml_dtypes.bfloat16)
    return wt


def stream_weights(nc, pool, w_dram, tag):
    """Rotating-chunk DMAs of one conv weight tensor, alternating between
    the sync and gpsimd DMA queues so chunk transfers overlap."""
    chunks = []
    c0 = 0
    for i, n in enumerate(CHUNKS):
        wt = pool.tile([128, max(CHUNKS), 2, 256], BF16, tag=f"{tag}wS",
                       name=f"{tag}wS{c0}")
        eng = nc.sync if i % 2 == 0 else nc.gpsimd
        eng.dma_start(wt[:, :n],
                      w_dram[c0:c0 + n].rearrange("t h c o -> c t h o"))
        chunks.append((c0, n, wt[:, :n]))
        c0 += n
    return chunks


def resident_weights(nc, pool, w_dram, tag, chunk=8):
    """One SBUF-resident tile holding all taps (chunked DMAs)."""
    wt = pool.tile([128, NTAPS, 2, 256], BF16, tag=f"{tag}wR", name=f"{tag}wR")
    chunks = []
    for c0 in range(0, NTAPS, chunk):
        n = min(chunk, NTAPS - c0)
        nc.sync.dma_start(wt[:, c0:c0 + n],
                          w_dram[c0:c0 + n].rearrange("t h c o -> c t h o"))
        chunks.append((c0, n, wt[:, c0:c0 + n]))
    return chunks


def conv_pass(nc, psum_pool, chunks, inputs, tag, ptag, ohs=(0, 1),
              filler=None):
    """inputs: list of [xh0, xh1] bf16 tiles [128, 8, 8, BL]."""
    ptags = [ptag] * len(inputs) if isinstance(ptag, str) else ptag
    ps = [{oh: [psum_pool.tile([128, 4, 8, BL], F32,
                               tag=f"{ptags[ii]}ps{oh}{bk}",
                               name=f"{tag}ps{ii}{oh}{bk}")
                for bk in range(2)] for oh in ohs}
          for ii in range(len(inputs))]
    total = {0: 0, 1: 0}
    for pcs in PIECES:
        for p in pcs:
            total[p[0]] += 2            # x2 c_halves
    done = {}
    for c0, n, wt in chunks:
        for tl in range(n):
            pieces = PIECES[c0 + tl]
            for oh in ohs:
                for ch in range(2):
                    lhsT = wt[:, tl, ch, oh * 128:(oh + 1) * 128]
                    for ii, xh in enumerate(inputs):
                        for (bk, irb, ar0, ph, ic0, ac0, wc) in pieces:
                            key = (ii, oh, bk)
                            cnt = done.get(key, 0)
                            done[key] = cnt + 1
                            rhs = xh[ch][:, ar0:ar0 + ph, ac0:ac0 + wc, :]
                            out = ps[ii][oh][bk][:, irb:irb + ph,
                                                 ic0:ic0 + wc, :]
                            nc.tensor.matmul(out, lhsT, rhs,
                                             start=(cnt == 0),
                                             stop=(cnt == total[bk] - 1))
        if filler is not None:
            filler(c0)
    return ps


def build_kernel():
    """Build the full per-core kernel (same NEFF on all 8 cores)."""
    nc = bacc.Bacc("TRN2", target_bir_lowering=False, debug=False,
                   num_devices=NCORES)
    dt = {}
    for nmm in ("q", "k", "v"):
        dt[f"x{nmm}"] = nc.dram_tensor(f"x{nmm}", [2, 128, 8, 8, BL], BF16,
                                       kind="ExternalInput")
        dt[f"w{nmm}"] = nc.dram_tensor(f"w{nmm}", [NTAPS, 2, 128, 256], BF16,
                                       kind="ExternalInput")
        dt[f"bias{nmm}"] = nc.dram_tensor(f"bias{nmm}", [2, 128], F32,
                                          kind="ExternalInput")
    dt["wo_t"] = nc.dram_tensor("wo_t", [2, 128, 256], BF16,
                                kind="ExternalInput")
    dt["bo"] = nc.dram_tensor("bo", [1, 256], BF16, kind="ExternalInput")
    dt["ones"] = nc.dram_tensor("ones", [1, 128], BF16, kind="ExternalInput")
    dt["ident"] = nc.dram_tensor("ident", [128, 128], BF16,
                                 kind="ExternalInput")
    dt["out"] = nc.dram_tensor("out", [8, 128, 256], F32,
                               kind="ExternalOutput")

    with TileContext(nc) as tc:
      with tc.tile_pool(name="persist", bufs=1) as pp:
        bias_t = {}
        for nmm in ("q", "k", "v"):
            bias_t[nmm] = pp.tile([128, 2], F32, name=f"bias{nmm}_t")
        ones_t = pp.tile([1, 128], BF16, name="ones_t")
        bo_t = pp.tile([1, 256], BF16, name="bo_t")
        ident_t = pp.tile([128, 128], BF16, name="ident_t")
        wo_tt = [pp.tile([128, 256], BF16, name=f"wo_tt{h}") for h in range(2)]

        def emit_persist_dmas():
            # on the gpsimd queue: off the weight-stream critical path
            for nmm in ("q", "k", "v"):
                nc.gpsimd.dma_start(bias_t[nmm][:],
                                    dt[f"bias{nmm}"].ap().rearrange(
                                        "h c -> c h"))
            nc.gpsimd.dma_start(ones_t[:], dt["ones"][:])
            nc.gpsimd.dma_start(bo_t[:], dt["bo"][:])
            nc.gpsimd.dma_start(ident_t[:], dt["ident"][:])
            for h in range(2):
                nc.gpsimd.dma_start(wo_tt[h][:], dt["wo_t"][h])

        # conv2 outputs [o, b, pix] -- live into attention
        hh = {}
        for nmm in ("q", "k", "v"):
            hh[nmm] = [pp.tile([128, BL, NPIX], BF16, name=f"h{nmm}{h}")
                       for h in range(2)]
        # inputs and conv1 outputs, all bf16
        xx, x1 = {}, {}
        for nmm in ("q", "k", "v"):
            xx[nmm] = [pp.tile([128, 8, 8, BL], BF16, name=f"x{nmm}{h}")
                       for h in range(2)]
            x1[nmm] = [pp.tile([128, 8, 8, BL], BF16, name=f"x1{nmm}{h}")
                       for h in range(2)]
        # attention SBUF tiles
        E_t = pp.tile([128, NB2, H, NPIX], BF16, name="E_t")
        VT = pp.tile([128, NB2, H, 33], BF16, name="VT")
        OA2 = pp.tile([128, NB2, 2, 128], BF16, name="OA2")
        concat = [pp.tile([128, BL, NPIX], BF16, name=f"concat{h}")
                  for h in range(2)]
        rcp = pp.tile([128, NB2, H], F32, name="rcp")
        out_sb = pp.tile([128, 8, 256], F32, name="out_sb")

        def load_x(nmm):
            for h in range(2):
                nc.gpsimd.dma_start(xx[nmm][h][:], dt[f"x{nmm}"][h])

        def copy_out_relu(ps, x1t, bias):
            for oh in range(2):
                for bk in range(2):
                    nc.scalar.activation(
                        x1t[oh][:, bk * 4:(bk + 1) * 4, :, :],
                        ps[oh][bk][:],
                        mybir.ActivationFunctionType.Relu,
                        bias=bias[:, oh:oh + 1])

        def copy_out_final(ps_oh, out_t_oh, bias, oh, engine="scalar"):
            for bk in range(2):
                src = ps_oh[bk][:].rearrange("c pr pc b -> c b (pr pc)")
                dst = out_t_oh[:, :, bk * 32:(bk + 1) * 32]
                if engine == "scalar":
                    nc.scalar.activation(
                        dst, src, mybir.ActivationFunctionType.Identity,
                        bias=bias[:, oh:oh + 1])
                else:
                    nc.vector.tensor_scalar_add(dst, src, bias[:, oh:oh + 1])

        with tc.tile_pool(name="cvps", bufs=1, space="PSUM") as cvpp:
            # ---- P1: q -> q1 (stream wq) ----
            with tc.tile_pool(name="wsq", bufs=4) as wsp:
                load_x('q')
                chq = stream_weights(nc, wsp, dt["wq"].ap(), "q")
                emit_persist_dmas()
                ps = conv_pass(nc, cvpp, chq, [xx['q']], tag="p1", ptag="cvA")
                copy_out_relu(ps[0], x1['q'], bias_t['q'])

            with tc.tile_pool(name="wkres", bufs=1) as wkp:
                # ---- P4: v -> v1 (stream wv); prefetch resident wk ----
                with tc.tile_pool(name="wsv1", bufs=4) as wsv:
                    load_x('v')
                    chv = stream_weights(nc, wsv, dt["wv"].ap(), "v1")
                    chk = resident_weights(nc, wkp, dt["wk"].ap(), "k")
                    load_x('k')
                    ps = conv_pass(nc, cvpp, chv, [xx['v']], tag="p4",
                                   ptag="cvB")
                    copy_out_relu(ps[0], x1['v'], bias_t['v'])

                # ---- P2: k -> k1 (wk resident) ----
                ps = conv_pass(nc, cvpp, chk, [xx['k']], tag="p2", ptag="cvA")
                copy_out_relu(ps[0], x1['k'], bias_t['k'])

                # ---- P5: v1 -> vh (re-stream wv) ----
                with tc.tile_pool(name="wsv2", bufs=4) as wsv2:
                    chv2 = stream_weights(nc, wsv2, dt["wv"].ap(), "v2")
                    ps = conv_pass(nc, cvpp, chv2, [x1['v']], tag="p5",
                                   ptag="cvB")
                    for oh in range(2):
                        copy_out_final(ps[0][oh], hh['v'][oh], bias_t['v'], oh)

                # V transposes for attention: DMA XBAR (runs during P3).
                # in vh[oh][:, 2b2:2b2+2, :] = [128=(4h,32dk), (2b,64pix)];
                # out partitions = (par, kpix), free = (h, dk).  The XBAR
                # needs a contiguous 2D dst, so stage then strided-copy.
                nc.vector.memset(VT[:, :, :, 32:33], 1.0)
                vstage = pp.tile([128, NB2, 2, 128], BF16, name="vstage")
                for b2 in range(NB2):
                    for oh in range(2):
                        nc.sync.dma_start_transpose(
                            vstage[:, b2, oh, :],
                            hh['v'][oh][:, 2 * b2:2 * b2 + 2, :].rearrange(
                                "c b p -> c (b p)"))
                        nc.vector.tensor_copy(
                            VT[:, b2, oh * 4:(oh + 1) * 4, 0:32],
                            vstage[:, b2, oh, :].rearrange(
                                "p (h d) -> p h d", d=32))

                # ---- P3: {k1, q1} -> {kh, qh}; oh halves split so the
                # first half's copy-outs + scores overlap the second half.
                kh, qh = hh['k'], hh['q']
                pst = {}
                PTAG = ["cvAps00", "cvAps01", "cvBps00", "cvBps01",
                        "cvAps10", "cvAps11", "cvBps10", "cvBps11"]

                def scores_half(oh, tags):
                    for hp in range(4):
                        h = oh * 4 + hp
                        pst[h] = cvpp.tile([128, NB2, 64], F32, tag=tags[hp],
                                           name=f"pst{h}")
                    for half in range(2):
                        for b2 in range(half * 4, half * 4 + 4):
                            for par in range(2):
                                b = 2 * b2 + par
                                for hp in range(4):
                                    h = oh * 4 + hp
                                    nc.tensor.matmul(
                                        pst[h][64 * par:64 * par + 64, b2, :],
                                        kh[oh][hp * 32:(hp + 1) * 32, b, :],
                                        qh[oh][hp * 32:(hp + 1) * 32, b, :],
                                        start=True, stop=True,
                                        tile_position=(32 * hp, 64 * par))
                        for hp in range(4):
                            h = oh * 4 + hp
                            nc.scalar.activation(
                                E_t[:, half * 4:half * 4 + 4, h, :],
                                pst[h][:, half * 4:half * 4 + 4, :],
                                mybir.ActivationFunctionType.Exp, scale=RS)

                ps = conv_pass(nc, cvpp, chk, [x1['k'], x1['q']], tag="p3a",
                               ptag=["cvA", "cvB"], ohs=(0,))
                copy_out_final(ps[0][0], kh[0], bias_t['k'], 0, "scalar")
                copy_out_final(ps[1][0], qh[0], bias_t['k'], 0, "vector")
                scores_half(0, PTAG[0:4])

                ps = conv_pass(nc, cvpp, chk, [x1['k'], x1['q']], tag="p3b",
                               ptag=["cvA", "cvB"], ohs=(1,))
                copy_out_final(ps[0][1], kh[1], bias_t['k'], 1, "scalar")
                copy_out_final(ps[1][1], qh[1], bias_t['k'], 1, "vector")
                scores_half(1, PTAG[4:8])

                # ---- attention AV + normalize + transpose + projection,
                # pipelined per batch-pair ----
                cslice = [concat[oh].rearrange("c b p -> c (b p)")
                          for oh in range(2)]

                def av(b2):
                    pso = cvpp.tile([128, H, 33], F32, tag=PTAG[b2 % 4],
                                    name=f"pso{b2}")
                    for h in range(H):
                        for par in range(2):
                            nc.tensor.matmul(
                                pso[64 * par:64 * par + 64, h, :],
                                E_t[64 * par:64 * par + 64, b2, h, :],
                                VT[64 * par:64 * par + 64, b2, h, :],
                                start=True, stop=True)
                    nc.vector.reciprocal(
                        rcp[:, b2, :],
                        pso[:, :, 32:33].rearrange("p h one -> p (h one)"))
                    nc.vector.tensor_mul(
                        OA2[:, b2, :, :].rearrange(
                            "p oh (hp d) -> p (oh hp) d", d=32),
                        pso[:, :, 0:32],
                        rcp[:, b2, :].unsqueeze(-1).broadcast_to(
                            [128, H, 32]))

                def out_pair(b2):
                    for oh in range(2):
                        pot = cvpp.tile([128, 128], BF16,
                                        tag=PTAG[4 + (2 * b2 + oh) % 2],
                                        name=f"pot{b2}{oh}")
                        nc.tensor.transpose(pot[:], OA2[:, b2, oh, :],
                                            ident_t[:])
                        nc.vector.tensor_copy(
                            concat[oh][:, 2 * b2:2 * b2 + 2, :].rearrange(
                                "c b p -> c (b p)"),
                            pot[:])
                    pspr = cvpp.tile([128, 256], F32, tag=PTAG[6 + b2 % 2],
                                     name=f"pspr{b2}")
                    for oh in range(2):
                        nc.tensor.matmul(
                            pspr[:], cslice[oh][:, b2 * 128:(b2 + 1) * 128],
                            wo_tt[oh][:], start=(oh == 0), stop=False)
                    nc.tensor.matmul(pspr[:], ones_t[:], bo_t[:],
                                     start=False, stop=True)
                    nc.vector.tensor_copy(out_sb[:, b2, :], pspr[:])
                    nc.sync.dma_start(dt["out"][b2], out_sb[:, b2, :])

                for b2 in range(NB2):
                    av(b2)
                    if b2 >= 1:
                        out_pair(b2 - 1)
                out_pair(NB2 - 1)
    nc.compile()
    return nc


def prep_static(wk, bk, wq, bq, wv, bv, wo, bo):
    """Host-side weight prep shared by all cores."""
    st = {}
    for nmm, w, b in (("q", wq, bq), ("k", wk, bk), ("v", wv, bv)):
        st[f"w{nmm}"] = prep_weights(np.asarray(w, np.float32))
        st[f"bias{nmm}"] = np.ascontiguousarray(
            np.asarray(b, np.float32).reshape(2, 128))
    st["wo_t"] = np.ascontiguousarray(
        np.asarray(wo, np.float32).T).reshape(2, 128, 256).astype(
        ml_dtypes.bfloat16)
    st["bo"] = np.asarray(bo, np.float32).reshape(1, 256).astype(
        ml_dtypes.bfloat16)
    st["ones"] = np.ones((1, 128), ml_dtypes.bfloat16)
    st["ident"] = np.eye(128, dtype=np.float32).astype(ml_dtypes.bfloat16)
    return st


def prep_core_x(x, core):
    """x: [B, 8, 8, D] -> this core's bf16 [2, 128, 8, 8, BL]."""
    xs = np.asarray(x[core * BL:(core + 1) * BL], np.float32)
    return np.ascontiguousarray(xs.transpose(3, 1, 2, 0)).reshape(
        2, 128, 8, 8, BL).astype(ml_dtypes.bfloat16)


def make_in_maps(q, k, v, st):
    in_maps = []
    for core in range(NCORES):
        m = dict(st)
        m["xq"] = prep_core_x(q, core)
        m["xk"] = prep_core_x(k, core)
        m["xv"] = prep_core_x(v, core)
        in_maps.append(m)
    return in_maps


def gather_out(results):
    """results: list of dicts with 'out' [8, 128, 256] -> [B, 8, 8, D]."""
    outs = [r["out"].reshape(BL, 8, 8, D) for r in results]
    return np.concatenate(outs, axis=0)


# ---------------------------------------------------------------------------
# Self-contained entry point: kernel(**inputs) -> full [128, 8, 8, 256] output
# ---------------------------------------------------------------------------
_NC_CACHE = None


def _get_nc():
    global _NC_CACHE
    if _NC_CACHE is None:
        _NC_CACHE = build_kernel()
    return _NC_CACHE


def kernel(q, k, v, wk, bk, wq, bq, wv, bv, wo, bo):
    nc = _get_nc()
    st = prep_static(wk, bk, wq, bq, wv, bv, wo, bo)
    in_maps = make_in_maps(np.asarray(q), np.asarray(k), np.asarray(v), st)
    res = bass_utils.run_bass_kernel_spmd(
        nc, in_maps, core_ids=list(range(NCORES)))
    return gather_out(res.results)
